# revision 1
# baseline (speedup 1.0000x reference)
"""Trainium2 Bass kernel for int8 GEMM + bias (IntLinear).

Computes y = x @ weight + bias with x:[8192,4096] int8, weight:[4096,4096] int8,
bias:[4096] int8 -> y:[8192,4096] int32.

Strategy
--------
- int8 values are exactly representable in bf16 (8-bit significand), and every
  product/partial sum of this GEMM stays far below 2^24 (measured max partial
  sum ~1.9M vs 16.7M), so a bf16 x bf16 matmul with fp32 PSUM accumulation
  reproduces the int32 result bit-exactly.
- Data-parallel shard: rows of x split across 8 NeuronCores (1024 rows each);
  weight replicated. Each core runs a tiled bf16 GEMM (XT stationary, W moving)
  and emits int32.
- Host side: transpose x -> [K, M] and convert both operands to bf16 (the
  PE needs the contraction dim on partitions for both operands; doing the
  transpose/convert in numpy keeps it off the device critical path).
- bias is added on host in int32 (exact; bias is int8).
"""

import numpy as np
import ml_dtypes

import concourse.bass  # noqa: F401  (registers engines)
import concourse.mybir as mybir
import concourse.tile as tile
from concourse import bacc
from concourse.bass_utils import run_bass_kernel_spmd
from concourse.kernels.tile_matmul import (
    composable_matmul_tile_kernel,
    dma_from_dram_kxm,
    dma_from_dram_kxn,
    dma_to_dram_mxn,
    k_pool_min_bufs,
)

M, K, N = 8192, 4096, 4096
N_CORES = 8
M_LOC = M // N_CORES

_compiled = None


def _build():
    global _compiled
    if _compiled is not None:
        return _compiled

    nc = bacc.Bacc("TRN2", target_bir_lowering=False, debug=False,
                   num_devices=N_CORES)
    xt = nc.dram_tensor("xt", [K, M_LOC], mybir.dt.bfloat16,
                        kind="ExternalInput").ap()
    w = nc.dram_tensor("w", [K, N], mybir.dt.bfloat16,
                       kind="ExternalInput").ap()
    y = nc.dram_tensor("y", [M_LOC, N], mybir.dt.int32,
                       kind="ExternalOutput").ap()

    with tile.TileContext(nc) as tc:
        tc.swap_default_side()
        num_bufs = k_pool_min_bufs(w, transpose_ap=False, max_tile_size=512)
        with (
            tc.tile_pool(name="kxm_pool", bufs=num_bufs) as kxm_pool,
            tc.tile_pool(name="kxn_pool", bufs=num_bufs) as kxn_pool,
        ):
            kxm_producer, kxm_shape = dma_from_dram_kxm(kxm_pool, xt)
            kxn_producer, kxn_shape = dma_from_dram_kxn(kxn_pool, w)
            mxn_consumer = dma_to_dram_mxn(y)
            composable_matmul_tile_kernel(
                tc=tc,
                kxm_shape=kxm_shape,
                kxn_shape=kxn_shape,
                output_type=mybir.dt.int32,
                kxm_producer=kxm_producer,
                kxn_producer=kxn_producer,
                mxn_consumer=mxn_consumer,
                MATMUL_FREE_DIM=512,
                MAX_TILE_SIZE=512,
                MAX_K_TILE_SIZE=512,
                cache_tiles=True,
                # 4 PSUM tags (one per m-subtile) x 2 bufs = all 8 banks,
                # so block N+1 accumulates into fresh banks while block N
                # drains.
                psum_n_bufs=2,
                temps_n_bufs=4,
            )
    nc.compile()
    _compiled = nc
    return nc


def _run(x, weight, trace=False, **spmd_kwargs):
    """Run the device GEMM. Returns (y_int32 [M,N], BassKernelResults)."""
    nc = _build()
    xt_bf16 = np.ascontiguousarray(x.T).astype(ml_dtypes.bfloat16)  # [K, M]
    w_bf16 = np.asarray(weight).astype(ml_dtypes.bfloat16)          # [K, N]
    in_maps = [
        {
            "xt": np.ascontiguousarray(xt_bf16[:, i * M_LOC:(i + 1) * M_LOC]),
            "w": w_bf16,
        }
        for i in range(N_CORES)
    ]
    res = run_bass_kernel_spmd(nc, in_maps, list(range(N_CORES)),
                               trace=trace, **spmd_kwargs)
    y = np.concatenate([res.results[i]["y"] for i in range(N_CORES)], axis=0)
    return y, res


def kernel(x, weight, bias):
    y, _ = _run(np.asarray(x), np.asarray(weight))
    return y + np.asarray(bias).astype(np.int32)

